# revision 17
# baseline (speedup 1.0000x reference)
"""BinaryNet (VGG-like binarized CNN) forward pass on 8 Trainium2 NeuronCores.

Data parallel: batch 256 sharded 32 images/core; weights/thresholds replicated.

Numerics: all hidden layers are exact-integer arithmetic (sign() activations and
sign() weights are +-1, exactly representable in fp8; PSUM accumulates in fp32),
so BN+sign folds into an integer threshold per channel computed on the host:
    sign((h - m) * rsqrt(v + eps) + b)  ==  sign(h - t),  t = m - b*sqrt(v+eps)
Binary conv/fc sums have even parity, so t can be snapped to an odd integer,
making the comparison exact and Sign(0) unreachable. Only conv1 (real-valued
input) runs in fp32. maxpool commutes with the monotone BN, so pooling happens
on raw integers before thresholding. The tiny final BN + softmax run on host in
fp32 (the device returns the 256x10 integer logits).

Binary convs use fp8 DoubleRow matmuls (2 MACs/cell/cycle):
  - conv1_2: vertical tap pairs (dy=0,1) via overlapping APs, row pitch 48
  - conv2_1: same with row pitch 32, per-image chunks
  - conv2_2: input-channel pairs, per-image interior chunks
  - conv3_x: input-channel pairs over contiguous padded-image windows (border
    outputs are garbage and never read)
  - fc5: input-channel-chunk pairs, weights streamed from HBM
DoubleRow pair steps must be multiples of 16 bytes (hardware constraint; a
non-aligned pair step hard-faults the device).
"""

import contextlib

import numpy as np
import ml_dtypes
import bass_rust

import concourse.bass as bass
import concourse.tile as tile
from concourse import bacc, mybir
from concourse.bass_utils import run_bass_kernel_spmd

F32 = mybir.dt.float32
F8 = mybir.dt.float8e4
NP8 = ml_dtypes.float8_e4m3
DR = mybir.MatmulPerfMode.DoubleRow

BN_EPS = 1e-3
N_CORES = 8
B = 32          # images per core

IM1 = 34 * 48        # conv1_2 input: 34 rows x pitch 48 (fp8, pair step 48)
IM2 = 18 * 32        # conv2_1 input: 18 rows x pitch 32
IM3 = 18 * 18        # conv2_2 input: 18x18 (cin-paired, no tap pairs)
IM4 = 10 * 10        # (unused in canvas mode)
CV_R, CV_C = 37, 73  # conv3 canvas: 4x8 grid of 8x8 images, shared zero borders
CV = 2704            # canvas flat size per cin-chunk, padded to %16
CSLACK = 80          # front/back slack for canvas window reads
XP_LEN = B * 34 * 34 + 128   # flat padded fp32 input per channel (+ tap slack)

# weight blob sizes (free-dim elements per partition)
WSZ = {"1_2": 3 * 2 * 128 + 3 * 128,
       "2_1": 2 * (3 * 2 * 128 + 3 * 128),
       "2_2": 2 * 9 * 2 * 128,
       "3_1": 4 * 9 * 2 * 128,
       "3_2": 4 * 9 * 2 * 2 * 128}
WOFF = {}
_off = 0
for _k in ("1_2", "2_1", "2_2", "3_1", "3_2"):
    WOFF[_k] = _off
    _off += WSZ[_k]
WC_TOT = _off


def _sgn(a):
    return np.where(a >= 0, np.float32(1.0), np.float32(-1.0))


def _thresh(p, name):
    m = np.asarray(p[name + "_mean"], np.float64)
    v = np.asarray(p[name + "_var"], np.float64)
    b = np.asarray(p[name + "_beta"], np.float64)
    return m - b * np.sqrt(v + BN_EPS)


def _odd_thresh(p, name):
    t = _thresh(p, name)
    return (2.0 * np.ceil(t / 2.0) - 1.0).astype(np.float32)


def _custom(apv, free_dims):
    """AP with explicit free [step, count] dims (keeps offset + partition)."""
    c = apv.copy()
    c.ap = bass_rust.VecI64Pair([list(list(apv.ap)[0])] + free_dims)
    return c


def build_program(repeat=1):
    nc = bacc.Bacc("TRN2", target_bir_lowering=False, debug=False,
                   num_devices=N_CORES)

    xp_d = nc.dram_tensor("xp", [3, XP_LEN], F32, kind="ExternalInput").ap()
    w1_d = nc.dram_tensor("w1", [27, 128], F32, kind="ExternalInput").ap()
    wc_d = nc.dram_tensor("wc", [128, WC_TOT], F8, kind="ExternalInput").ap()
    wf5_d = nc.dram_tensor("wf5", [128, 64 * 1024], F8, kind="ExternalInput").ap()
    wf6_d = nc.dram_tensor("wf6", [128, 8 * 1024], F8, kind="ExternalInput").ap()
    wf7_d = nc.dram_tensor("wf7", [128, 8 * 10], F8, kind="ExternalInput").ap()
    thr_d = nc.dram_tensor("thr", [128, 30], F32, kind="ExternalInput").ap()
    id_d = nc.dram_tensor("ident", [32, 32], F32, kind="ExternalInput").ap()
    out_d = nc.dram_tensor("out", [B, 10], F32, kind="ExternalOutput").ap()

    tcol = {"1_1": 0, "1_2": 1, "2_1": 2, "2_2": 4, "3_1": 6, "3_2": 10,
            "5": 14, "6": 22}

    with tile.TileContext(nc) as tc:
        with contextlib.ExitStack() as ctx:
            if repeat > 1:
                ctx.enter_context(tc.For_i(0, repeat, 1))
            const_p = ctx.enter_context(tc.tile_pool(name="const", bufs=1))
            acts_a = ctx.enter_context(tc.tile_pool(name="actsA", bufs=1))
            acts_b = ctx.enter_context(tc.tile_pool(name="actsB", bufs=1))
            xcol_p = ctx.enter_context(tc.tile_pool(name="xcol", bufs=2))
            wconv_p = ctx.enter_context(tc.tile_pool(name="wconv", bufs=2))
            wf5_p = ctx.enter_context(tc.tile_pool(name="wf5s", bufs=24))
            tmp_p = ctx.enter_context(tc.tile_pool(name="tmp", bufs=6))
            fc_p = ctx.enter_context(tc.tile_pool(name="fc", bufs=2))
            ps_conv = ctx.enter_context(
                tc.tile_pool(name="psc", bufs=4, space="PSUM"))
            ps_fc = ctx.enter_context(
                tc.tile_pool(name="psfc", bufs=2, space="PSUM"))
            ps_tp = ctx.enter_context(
                tc.tile_pool(name="pstp", bufs=2, space="PSUM"))

            thr_sb = const_p.tile([128, 30], F32)
            nc.sync.dma_start(thr_sb[:], thr_d[:])
            id_sb = const_p.tile([32, 32], F32)
            nc.sync.dma_start(id_sb[:], id_d[:])
            w1_sb = const_p.tile([27, 128], F32)
            nc.sync.dma_start(w1_sb[:], w1_d[:])
            wf6_sb = const_p.tile([128, 8 * 1024], F8)
            nc.sync.dma_start(wf6_sb[:], wf6_d[:])
            wf7_sb = const_p.tile([128, 80], F8)
            nc.sync.dma_start(wf7_sb[:], wf7_d[:])

            def bias(name, od=0):
                c = tcol[name] + od
                return thr_sb[:, c:c + 1]

            # ---------------- L1: conv1_1, fp32, K=27 im2col ----------------
            s1 = acts_a.tile([128, B * IM1], F8, tag="A")
            s1f0 = s1[:].rearrange("c (b n) -> c b n", b=B)
            nc.gpsimd.memset(s1f0[:, :, 0:34], 0.0)              # row 0
            nc.gpsimd.memset(s1f0[:, :, 33 * 48:33 * 48 + 34], 0.0)  # row 33
            nc.gpsimd.memset(_custom(s1[:, 48:49], [[IM1, B], [48, 32]]), 0.0)
            nc.gpsimd.memset(_custom(s1[:, 48 + 33:48 + 34],
                                     [[IM1, B], [48, 32]]), 0.0)
            s1_v = s1[:].rearrange("c (b h w) -> c b h w", b=B, h=34, w=48)
            GRP = 2
            for g in range(B // GRP):
                xcol = xcol_p.tile([128, GRP * 1156], F32, tag="xcol")
                for t in range(9):
                    dy, dx = t // 3, t % 3
                    off = g * GRP * 1156 + dy * 34 + dx
                    nc.sync.dma_start(xcol[3 * t:3 * t + 3, :],
                                      xp_d[:, off:off + GRP * 1156])
                xc_v = xcol[:].rearrange("c (b n) -> c b n", b=GRP)
                for li in range(GRP):
                    bi = g * GRP + li
                    for half in range(2):
                        ps = ps_conv.tile([128, 512], F32, tag="ps")
                        rhs = xc_v[0:27, li, half * 544:half * 544 + 544] \
                            .rearrange("c (h w) -> c h w", h=16, w=34)[:, :, 0:32]
                        nc.tensor.matmul(
                            ps[:].rearrange("c (h w) -> c h w", h=16, w=32),
                            w1_sb[:], rhs, start=True, stop=True)
                        dst = s1_v[:, bi, 1 + half * 16:1 + half * 16 + 16, 1:33]
                        nc.scalar.activation(
                            dst, ps[:].rearrange("c (h w) -> c h w", h=16, w=32),
                            mybir.ActivationFunctionType.Sign, bias=bias("1_1"))

            # ---------------- L2: conv1_2 (128->128, 32x32, pool) -----------
            # DR over vertical tap pairs (dy=0,1); dy=2 as normal matmuls.
            w2 = wconv_p.tile([128, WSZ["1_2"]], F8, tag="w")
            nc.sync.dma_start(w2[:], wc_d[:, WOFF["1_2"]:WOFF["1_2"] + WSZ["1_2"]])
            s2 = acts_b.tile([128, B * IM2], F8, tag="B")
            s2f0 = s2[:].rearrange("c (b n) -> c b n", b=B)
            nc.gpsimd.memset(s2f0[:, :, 0:18], 0.0)              # row 0
            nc.gpsimd.memset(s2f0[:, :, 17 * 32:17 * 32 + 18], 0.0)  # row 17
            nc.gpsimd.memset(_custom(s2[:, 32:33], [[IM2, B], [32, 16]]), 0.0)
            nc.gpsimd.memset(_custom(s2[:, 32 + 17:32 + 18],
                                     [[IM2, B], [32, 16]]), 0.0)
            s2_v = s2[:].rearrange("c (b h w) -> c b h w", b=B, h=18, w=32)
            for g in range(B):
                for half in range(2):
                    y0 = half * 16
                    ps = ps_conv.tile([128, 512], F32, tag="ps")
                    psv = ps[:].rearrange("c (h w) -> c h w", h=16, w=32)
                    for dx in range(3):
                        base = s1[:, g * IM1 + y0 * 48 + dx:
                                  g * IM1 + y0 * 48 + dx + 1]
                        rhs = _custom(base, [[48, 2], [48, 16], [1, 32]])
                        nc.tensor.matmul(
                            psv, w2[:, dx * 256:dx * 256 + 256]
                            .rearrange("c (t m) -> c t m", t=2),
                            rhs, start=(dx == 0), stop=False, perf_mode=DR)
                    for dx in range(3):
                        rhs = s1_v[:, g, y0 + 2:y0 + 18, dx:dx + 32]
                        nc.tensor.matmul(
                            psv, w2[:, 768 + dx * 128:768 + dx * 128 + 128],
                            rhs, start=False, stop=(dx == 2))
                    # pool 2x2 on integers, then threshold-sign
                    t1 = tmp_p.tile([128, 256], F32, tag="t1")
                    nc.vector.tensor_reduce(
                        t1[:].rearrange("c (m x) -> c m x", m=16, x=16),
                        ps[:].rearrange("c (m x t) -> c m x t", m=16, x=16, t=2),
                        mybir.AxisListType.X, mybir.AluOpType.max)
                    t2 = tmp_p.tile([128, 128], F32, tag="t2")
                    nc.vector.tensor_reduce(
                        t2[:].rearrange("c (m x) -> c m x", m=8, x=16),
                        t1[:].rearrange("c (m t x) -> c m x t", m=8, t=2, x=16),
                        mybir.AxisListType.X, mybir.AluOpType.max)
                    dst = s2_v[:, g, 1 + half * 8:1 + half * 8 + 8, 1:17]
                    nc.scalar.activation(
                        dst, t2[:].rearrange("c (h w) -> c h w", h=8, w=16),
                        mybir.ActivationFunctionType.Sign, bias=bias("1_2"))

            # ---------------- L3: conv2_1 (128->256, 16x16) -----------------
            w3 = wconv_p.tile([128, WSZ["2_1"]], F8, tag="w")
            nc.sync.dma_start(w3[:], wc_d[:, WOFF["2_1"]:WOFF["2_1"] + WSZ["2_1"]])
            s3 = acts_a.tile([128, 2 * B * IM3], F8, tag="A")
            s3f0 = s3[:].rearrange("c (q n) -> c q n", q=2 * B)
            nc.gpsimd.memset(s3f0[:, :, 0:18], 0.0)
            nc.gpsimd.memset(s3f0[:, :, 17 * 18:18 * 18], 0.0)
            nc.gpsimd.memset(_custom(s3[:, 18:19], [[IM3, 2 * B], [18, 16]]), 0.0)
            nc.gpsimd.memset(_custom(s3[:, 18 + 17:18 + 18],
                                     [[IM3, 2 * B], [18, 16]]), 0.0)
            s3_v = s3[:].rearrange("c (q b h w) -> c q b h w",
                                   q=2, b=B, h=18, w=18)
            s2f = s2[:]
            for g in range(B):
                for od in range(2):
                    wo = od * (3 * 2 * 128 + 3 * 128)
                    ps = ps_conv.tile([128, 256], F32, tag="ps")
                    psv = ps[:].rearrange("c (h w) -> c h w", h=16, w=16)
                    for dx in range(3):
                        base = s2f[:, g * IM2 + dx:g * IM2 + dx + 1]
                        rhs = _custom(base, [[32, 2], [32, 16], [1, 16]])
                        nc.tensor.matmul(
                            psv, w3[:, wo + dx * 256:wo + dx * 256 + 256]
                            .rearrange("c (t m) -> c t m", t=2),
                            rhs, start=(dx == 0), stop=False, perf_mode=DR)
                    for dx in range(3):
                        rhs = s2_v[:, g, 2:18, dx:dx + 16]
                        nc.tensor.matmul(
                            psv, w3[:, wo + 768 + dx * 128:wo + 768 + dx * 128 + 128],
                            rhs, start=False, stop=(dx == 2))
                    dst = s3_v[:, od, g, 1:17, 1:17]
                    nc.scalar.activation(
                        dst, psv, mybir.ActivationFunctionType.Sign,
                        bias=bias("2_1", od))

            # ---------------- L4: conv2_2 (256->256, 16x16, pool) -----------
            # DR over cin pairs; per-image interior chunks N=256.
            w4 = wconv_p.tile([128, WSZ["2_2"]], F8, tag="w")
            nc.sync.dma_start(w4[:], wc_d[:, WOFF["2_2"]:WOFF["2_2"] + WSZ["2_2"]])
            s4 = acts_b.tile([128, 2 * CSLACK + 2 * CV], F8, tag="B")

            def canvas_memset(s, ncd):
                L = 2 * CSLACK + ncd * CV
                nc.gpsimd.memset(s[:, 0:CSLACK], 0.0)
                nc.gpsimd.memset(s[:, L - CSLACK:L], 0.0)
                for cd in range(ncd):
                    b0 = CSLACK + cd * CV
                    # 5 border rows (0,9,18,27,36), 9 border cols, 3 pad elems
                    nc.gpsimd.memset(
                        _custom(s[:, b0:b0 + 1], [[9 * CV_C, 5], [1, CV_C]]), 0.0)
                    nc.gpsimd.memset(
                        _custom(s[:, b0:b0 + 1], [[CV_C, CV_R], [9, 9]]), 0.0)
                    nc.gpsimd.memset(s[:, b0 + 2701:b0 + 2704], 0.0)

            canvas_memset(s4, 2)
            for g in range(B):
                for od in range(2):
                    ps = ps_conv.tile([128, 256], F32, tag="ps")
                    psv = ps[:].rearrange("c (h w) -> c h w", h=16, w=16)
                    for t in range(9):
                        dy, dx = t // 3, t % 3
                        wo = (od * 9 + t) * 256
                        rhs = s3_v[:, :, g, dy:dy + 16, dx:dx + 16]
                        nc.tensor.matmul(
                            psv, w4[:, wo:wo + 256]
                            .rearrange("c (t m) -> c t m", t=2),
                            rhs, start=(t == 0), stop=(t == 8), perf_mode=DR)
                    t1 = tmp_p.tile([128, 128], F32, tag="t1")
                    nc.vector.tensor_reduce(
                        t1[:, 0:128].rearrange("c (m x) -> c m x", m=16, x=8),
                        ps[:].rearrange("c (m x t) -> c m x t", m=16, x=8, t=2),
                        mybir.AxisListType.X, mybir.AluOpType.max)
                    t2 = tmp_p.tile([128, 64], F32, tag="t2")
                    nc.vector.tensor_reduce(
                        t2[:, 0:64].rearrange("c (m x) -> c m x", m=8, x=8),
                        t1[:, 0:128].rearrange("c (m t x) -> c m x t",
                                               m=8, t=2, x=8),
                        mybir.AxisListType.X, mybir.AluOpType.max)
                    ir, ic = g // 8, g % 8
                    b4 = (CSLACK + od * CV + (1 + 9 * ir) * CV_C + 1 + 9 * ic)
                    dst = _custom(s4[:, b4:b4 + 1], [[CV_C, 8], [1, 8]])
                    nc.scalar.activation(
                        dst, t2[:, 0:64].rearrange("c (h w) -> c h w", h=8, w=8),
                        mybir.ActivationFunctionType.Sign, bias=bias("2_2", od))

            # ---------------- L5: conv3_1 (256->512, 8x8) -------------------
            # DR over cin pairs; 6-canvas-row chunks N=438 (borders junk).
            w5 = wconv_p.tile([128, WSZ["3_1"]], F8, tag="w")
            nc.sync.dma_start(w5[:], wc_d[:, WOFF["3_1"]:WOFF["3_1"] + WSZ["3_1"]])
            s5 = acts_a.tile([128, 2 * CSLACK + 4 * CV], F8, tag="A")
            canvas_memset(s5, 4)
            SEGS = {0: [(1, 6)], 1: [(7, 2), (10, 3)], 2: [(13, 5)],
                    3: [(19, 6)], 4: [(25, 2), (28, 3)], 5: [(31, 5)]}
            for c6 in range(6):
                r0 = 1 + 6 * c6
                for od in range(4):
                    ps = ps_conv.tile([128, 438], F32, tag="ps")
                    for t in range(9):
                        dy, dx = t // 3, t % 3
                        wo = (od * 9 + t) * 256
                        boff = (CSLACK + r0 * CV_C + (dy - 1) * CV_C + (dx - 1))
                        rhs = _custom(s4[:, boff:boff + 1], [[CV, 2], [1, 438]])
                        nc.tensor.matmul(
                            ps[:], w5[:, wo:wo + 256]
                            .rearrange("c (t m) -> c t m", t=2),
                            rhs, start=(t == 0), stop=(t == 8), perf_mode=DR)
                    for rs, nr in SEGS[c6]:
                        src_ = _custom(ps[:, (rs - r0) * CV_C + 1:
                                       (rs - r0) * CV_C + 2],
                                       [[CV_C, nr], [9, 8], [1, 8]])
                        dpos = CSLACK + od * CV + rs * CV_C + 1
                        dst = _custom(s5[:, dpos:dpos + 1],
                                      [[CV_C, nr], [9, 8], [1, 8]])
                        nc.scalar.activation(
                            dst, src_, mybir.ActivationFunctionType.Sign,
                            bias=bias("3_1", od))

            # ---------------- L6: conv3_2 (512->512, 8x8, pool) -------------
            # 4-canvas-row chunks aligned to image rows, N=292.
            w6 = wconv_p.tile([128, WSZ["3_2"]], F8, tag="w")
            nc.sync.dma_start(w6[:], wc_d[:, WOFF["3_2"]:WOFF["3_2"] + WSZ["3_2"]])
            s6 = acts_b.tile([128, 4 * B * 16], F8, tag="B")
            for ir in range(4):
                for h in range(2):
                    r0 = 9 * ir + 1 + 4 * h
                    for od in range(4):
                        ps = ps_conv.tile([128, 292], F32, tag="ps")
                        mi = 0
                        for pp in range(2):
                            for t in range(9):
                                dy, dx = t // 3, t % 3
                                wo = ((od * 9 + t) * 2 + pp) * 256
                                boff = (CSLACK + pp * 2 * CV + r0 * CV_C
                                        + (dy - 1) * CV_C + (dx - 1))
                                rhs = _custom(s5[:, boff:boff + 1],
                                              [[CV, 2], [1, 292]])
                                nc.tensor.matmul(
                                    ps[:], w6[:, wo:wo + 256]
                                    .rearrange("c (t m) -> c t m", t=2),
                                    rhs, start=(mi == 0), stop=(mi == 17),
                                    perf_mode=DR)
                                mi += 1
                        # pool 2x2: per-row x-pairs, then y-pairs merged
                        t1 = tmp_p.tile([128, 128], F32, tag="t1")
                        for j in range(4):
                            src1 = _custom(ps[:, j * CV_C + 1:j * CV_C + 2],
                                           [[9, 8], [2, 4], [1, 2]])
                            nc.vector.tensor_reduce(
                                t1[:, j * 32:j * 32 + 32]
                                .rearrange("c (m x) -> c m x", m=8, x=4),
                                src1, mybir.AxisListType.X, mybir.AluOpType.max)
                        t2 = tmp_p.tile([128, 64], F32, tag="t2")
                        nc.vector.tensor_reduce(
                            t2[:, 0:64].rearrange("c (m x) -> c m x", m=2, x=32),
                            t1[:].rearrange("c (m t x) -> c m x t",
                                            m=2, t=2, x=32),
                            mybir.AxisListType.X, mybir.AluOpType.max)
                        base6 = (od * B + ir * 8) * 16 + 8 * h
                        dst = _custom(s6[:, base6:base6 + 1],
                                      [[4, 2], [16, 8], [1, 4]])
                        nc.scalar.activation(
                            dst, t2[:, 0:64].rearrange("c (r i x) -> c r i x",
                                                       r=2, i=8, x=4),
                            mybir.ActivationFunctionType.Sign,
                            bias=bias("3_2", od))

            # ---------------- FC head ----------------
            # fc5 with DR: lhsT = s6 cd-pairs [128,2,32], rhs = streamed W5
            ps5a = ps_fc.tile([32, 512], F32, tag="fc")
            ps5b = ps_fc.tile([32, 512], F32, tag="fc")
            s6_v = s6[:].rearrange("c (p t b n) -> c p t b n", p=2, t=2, b=B)
            for kk in range(32):
                cdp, yx = kk // 16, kk % 16
                w5t = wf5_p.tile([128, 2048], F8, tag="w5")
                nc.sync.dma_start(w5t[:], wf5_d[:, kk * 2048:(kk + 1) * 2048])
                lhsT = s6_v[:, cdp, :, :, yx]        # [128, 2, 32]
                w5v = w5t[:].rearrange("c (t n) -> c t n", t=2)
                nc.tensor.matmul(ps5a[:], lhsT, w5v[:, :, 0:512],
                                 start=(kk == 0), stop=(kk == 31), perf_mode=DR)
                nc.tensor.matmul(ps5b[:], lhsT, w5v[:, :, 512:1024],
                                 start=(kk == 0), stop=(kk == 31), perf_mode=DR)
            h5 = fc_p.tile([32, 1024], F32, tag="h")
            nc.vector.tensor_copy(h5[:, 0:512], ps5a[:])
            nc.vector.tensor_copy(h5[:, 512:1024], ps5b[:])

            s5t = fc_p.tile([128, 8 * 32], F8, tag="st")
            for c in range(8):
                pt = ps_tp.tile([128, 32], F32, tag="tp")
                nc.tensor.transpose(pt[:], h5[:, c * 128:(c + 1) * 128], id_sb[:])
                nc.scalar.activation(
                    s5t[:, c * 32:(c + 1) * 32], pt[:],
                    mybir.ActivationFunctionType.Sign,
                    bias=thr_sb[:, tcol["5"] + c:tcol["5"] + c + 1])

            ps6a = ps_fc.tile([32, 512], F32, tag="fc")
            ps6b = ps_fc.tile([32, 512], F32, tag="fc")
            for c in range(8):
                lhsT = s5t[:, c * 32:(c + 1) * 32]
                nc.tensor.matmul(ps6a[:], lhsT, wf6_sb[:, c * 1024:c * 1024 + 512],
                                 start=(c == 0), stop=(c == 7))
                nc.tensor.matmul(ps6b[:], lhsT,
                                 wf6_sb[:, c * 1024 + 512:c * 1024 + 1024],
                                 start=(c == 0), stop=(c == 7))
            h6 = fc_p.tile([32, 1024], F32, tag="h")
            nc.vector.tensor_copy(h6[:, 0:512], ps6a[:])
            nc.vector.tensor_copy(h6[:, 512:1024], ps6b[:])

            s6t = fc_p.tile([128, 8 * 32], F8, tag="st")
            for c in range(8):
                pt = ps_tp.tile([128, 32], F32, tag="tp")
                nc.tensor.transpose(pt[:], h6[:, c * 128:(c + 1) * 128], id_sb[:])
                nc.scalar.activation(
                    s6t[:, c * 32:(c + 1) * 32], pt[:],
                    mybir.ActivationFunctionType.Sign,
                    bias=thr_sb[:, tcol["6"] + c:tcol["6"] + c + 1])

            ps7 = ps_tp.tile([32, 10], F32, tag="tp")
            for c in range(8):
                nc.tensor.matmul(ps7[:], s6t[:, c * 32:(c + 1) * 32],
                                 wf7_sb[:, c * 10:(c + 1) * 10],
                                 start=(c == 0), stop=(c == 7))
            o_sb = const_p.tile([32, 10], F32)
            nc.vector.tensor_copy(o_sb[:], ps7[:])
            nc.sync.dma_start(out_d[:], o_sb[:])

    nc.compile()
    return nc


def prep_inputs(x, params):
    x = np.asarray(x, np.float32)
    p = {k: np.asarray(v) for k, v in params.items()}

    xs = x.reshape(N_CORES, B, 32, 32, 3)
    xp = np.zeros((N_CORES, 3, B, 34, 34), np.float32)
    xp[:, :, :, 1:33, 1:33] = xs.transpose(0, 4, 1, 2, 3)
    xp_flat = np.zeros((N_CORES, 3, XP_LEN), np.float32)
    xp_flat[:, :, :B * 1156] = xp.reshape(N_CORES, 3, -1)

    w1 = np.ascontiguousarray(
        _sgn(p["w_conv_1_1"]).reshape(27, 128)).astype(np.float32)

    wc = np.zeros((128, WC_TOT), NP8)

    def put(name, arr):
        wc[:, WOFF[name]:WOFF[name] + WSZ[name]] = \
            np.ascontiguousarray(arr.reshape(128, -1)).astype(NP8)

    # conv1_2: [dy,dx,cm,om] -> DR [cm,(dx,2,om)] + N [cm,(dx,om)]
    a = _sgn(p["w_conv_1_2"]).reshape(3, 3, 128, 128)
    put("1_2", np.concatenate(
        [a[0:2].transpose(2, 1, 0, 3).reshape(128, -1),
         a[2].transpose(1, 0, 2).reshape(128, -1)], axis=1))
    # conv2_1: [dy,dx,cm,od,om] -> per od: DR + N
    a = _sgn(p["w_conv_2_1"]).reshape(3, 3, 128, 2, 128)
    blocks = []
    for od in range(2):
        blocks.append(a[0:2, :, :, od].transpose(2, 1, 0, 3).reshape(128, -1))
        blocks.append(a[2, :, :, od].transpose(1, 0, 2).reshape(128, -1))
    put("2_1", np.concatenate(blocks, axis=1))
    # conv2_2: [dy,dx,cd2,cm,od2,om] -> [cm,(od,t,cd,om)]
    a = _sgn(p["w_conv_2_2"]).reshape(3, 3, 2, 128, 2, 128)
    put("2_2", a.transpose(3, 4, 0, 1, 2, 5).reshape(128, -1))
    # conv3_1: [dy,dx,cd2,cm,od4,om] -> [cm,(od,t,cd,om)]
    a = _sgn(p["w_conv_3_1"]).reshape(3, 3, 2, 128, 4, 128)
    put("3_1", a.transpose(3, 4, 0, 1, 2, 5).reshape(128, -1))
    # conv3_2: [dy,dx,pp2,i2,cm,od4,om] -> [cm,(od,t,pp,i,om)]
    a = _sgn(p["w_conv_3_2"]).reshape(3, 3, 2, 2, 128, 4, 128)
    put("3_2", a.transpose(4, 5, 0, 1, 2, 3, 6).reshape(128, -1))

    # fc5: rows f=(y,x,cd,cm); pair (2cdp, 2cdp+1) -> [cm,(cdp,y,x,i,o)]
    w5 = _sgn(p["w_fc_5"]).reshape(4, 4, 2, 2, 128, 1024)  # (y,x,cdp,i,cm,o)
    wf5 = np.ascontiguousarray(
        w5.transpose(4, 2, 0, 1, 3, 5).reshape(128, -1)).astype(NP8)
    w6 = _sgn(p["w_fc_6"]).reshape(8, 128, 1024)
    wf6 = np.ascontiguousarray(w6.transpose(1, 0, 2).reshape(128, -1)).astype(NP8)
    w7 = _sgn(p["w_fc_7"]).reshape(8, 128, 10)
    wf7 = np.ascontiguousarray(w7.transpose(1, 0, 2).reshape(128, -1)).astype(NP8)

    thr = np.zeros((128, 30), np.float32)
    thr[:, 0] = -_thresh(p, "bn_1_1").astype(np.float32)
    thr[:, 1] = -_odd_thresh(p, "bn_1_2")
    thr[:, 2:4] = -_odd_thresh(p, "bn_2_1").reshape(2, 128).T
    thr[:, 4:6] = -_odd_thresh(p, "bn_2_2").reshape(2, 128).T
    thr[:, 6:10] = -_odd_thresh(p, "bn_3_1").reshape(4, 128).T
    thr[:, 10:14] = -_odd_thresh(p, "bn_3_2").reshape(4, 128).T
    thr[:, 14:22] = -_odd_thresh(p, "bn_5").reshape(8, 128).T
    thr[:, 22:30] = -_odd_thresh(p, "bn_6").reshape(8, 128).T

    ident = np.eye(32, dtype=np.float32)

    shared = {"w1": w1, "wc": wc, "wf5": wf5, "wf6": wf6, "wf7": wf7,
              "thr": thr, "ident": ident}
    in_maps = [dict(shared, xp=xp_flat[c]) for c in range(N_CORES)]
    return in_maps


def postprocess(h7_all, params):
    p = params
    m = np.asarray(p["bn_7_mean"], np.float32)
    v = np.asarray(p["bn_7_var"], np.float32)
    b = np.asarray(p["bn_7_beta"], np.float32)
    y = (h7_all - m) * (np.float32(1.0) / np.sqrt(v + np.float32(BN_EPS))) + b
    y = y - y.max(axis=-1, keepdims=True)
    e = np.exp(y)
    return (e / e.sum(axis=-1, keepdims=True)).astype(np.float32)


_NC_CACHE = {}


def get_nc():
    if "nc" not in _NC_CACHE:
        _NC_CACHE["nc"] = build_program()
    return _NC_CACHE["nc"]


def kernel(x, params):
    params = {k: np.asarray(v) for k, v in params.items()}
    nc = get_nc()
    in_maps = prep_inputs(x, params)
    res = run_bass_kernel_spmd(nc, in_maps, list(range(N_CORES)))
    h7 = np.concatenate([res.results[c]["out"] for c in range(N_CORES)], axis=0)
    return postprocess(h7, params)


# revision 18
# speedup vs baseline: 1.0928x; 1.0928x over previous
"""BinaryNet (VGG-like binarized CNN) forward pass on 8 Trainium2 NeuronCores.

Data parallel: batch 256 sharded 32 images/core; weights/thresholds replicated.

Numerics: all hidden layers are exact-integer arithmetic (sign() activations and
sign() weights are +-1, exactly representable in fp8; PSUM accumulates in fp32),
so BN+sign folds into an integer threshold per channel computed on the host:
    sign((h - m) * rsqrt(v + eps) + b)  ==  sign(h - t),  t = m - b*sqrt(v+eps)
Binary conv/fc sums have even parity, so t can be snapped to an odd integer,
making the comparison exact and Sign(0) unreachable. Only conv1 (real-valued
input) runs in fp32. maxpool commutes with the monotone BN, so pooling happens
on raw integers before thresholding. The tiny final BN + softmax run on host in
fp32 (the device returns the 256x10 integer logits).

Binary convs use fp8 DoubleRow matmuls (2 MACs/cell/cycle):
  - conv1_2: vertical tap pairs (dy=0,1) via overlapping APs, row pitch 48
  - conv2_1: same with row pitch 32, per-image chunks
  - conv2_2: input-channel pairs, per-image interior chunks
  - conv3_x: input-channel pairs over contiguous padded-image windows (border
    outputs are garbage and never read)
  - fc5: input-channel-chunk pairs, weights streamed from HBM
DoubleRow pair steps must be multiples of 16 bytes (hardware constraint; a
non-aligned pair step hard-faults the device).
"""

import contextlib

import numpy as np
import ml_dtypes
import bass_rust

import concourse.bass as bass
import concourse.tile as tile
from concourse import bacc, mybir
from concourse.bass_utils import run_bass_kernel_spmd

F32 = mybir.dt.float32
F8 = mybir.dt.float8e4
NP8 = ml_dtypes.float8_e4m3
DR = mybir.MatmulPerfMode.DoubleRow

BN_EPS = 1e-3
N_CORES = 8
B = 32          # images per core

IM1 = 34 * 48        # conv1_2 input: 34 rows x pitch 48 (fp8, pair step 48)
IM2 = 18 * 32        # conv2_1 input: 18 rows x pitch 32
IM3 = 18 * 18        # conv2_2 input: 18x18 (cin-paired, no tap pairs)
IM4 = 10 * 10        # (unused in canvas mode)
CV_R, CV_C = 37, 73  # conv3 canvas: 4x8 grid of 8x8 images, shared zero borders
CV = 2704            # canvas flat size per cin-chunk, padded to %16
CSLACK = 80          # front/back slack for canvas window reads
XP_LEN = B * 34 * 34 + 128   # flat padded fp32 input per channel (+ tap slack)

# weight blob sizes (free-dim elements per partition)
WSZ = {"1_2": 3 * 2 * 128 + 3 * 128,
       "2_1": 2 * (3 * 2 * 128 + 3 * 128),
       "2_2": 2 * 9 * 2 * 128,
       "3_1": 4 * 9 * 2 * 128,
       "3_2": 4 * 9 * 2 * 2 * 128}
WOFF = {}
_off = 0
for _k in ("1_2", "2_1", "2_2", "3_1", "3_2"):
    WOFF[_k] = _off
    _off += WSZ[_k]
WC_TOT = _off


def _sgn(a):
    return np.where(a >= 0, np.float32(1.0), np.float32(-1.0))


def _thresh(p, name):
    m = np.asarray(p[name + "_mean"], np.float64)
    v = np.asarray(p[name + "_var"], np.float64)
    b = np.asarray(p[name + "_beta"], np.float64)
    return m - b * np.sqrt(v + BN_EPS)


def _odd_thresh(p, name):
    t = _thresh(p, name)
    return (2.0 * np.ceil(t / 2.0) - 1.0).astype(np.float32)


def _custom(apv, free_dims):
    """AP with explicit free [step, count] dims (keeps offset + partition)."""
    c = apv.copy()
    c.ap = bass_rust.VecI64Pair([list(list(apv.ap)[0])] + free_dims)
    return c


def build_program(repeat=1):
    nc = bacc.Bacc("TRN2", target_bir_lowering=False, debug=False,
                   num_devices=N_CORES)

    xp_d = nc.dram_tensor("xp", [3, XP_LEN], F32, kind="ExternalInput").ap()
    w1_d = nc.dram_tensor("w1", [27, 128], F32, kind="ExternalInput").ap()
    wc_d = nc.dram_tensor("wc", [128, WC_TOT], F8, kind="ExternalInput").ap()
    wf5_d = nc.dram_tensor("wf5", [128, 64 * 1024], F8, kind="ExternalInput").ap()
    wf6_d = nc.dram_tensor("wf6", [128, 8 * 1024], F8, kind="ExternalInput").ap()
    wf7_d = nc.dram_tensor("wf7", [128, 8 * 10], F8, kind="ExternalInput").ap()
    thr_d = nc.dram_tensor("thr", [128, 30], F32, kind="ExternalInput").ap()
    id_d = nc.dram_tensor("ident", [32, 32], F32, kind="ExternalInput").ap()
    out_d = nc.dram_tensor("out", [B, 10], F32, kind="ExternalOutput").ap()

    tcol = {"1_1": 0, "1_2": 1, "2_1": 2, "2_2": 4, "3_1": 6, "3_2": 10,
            "5": 14, "6": 22}

    with tile.TileContext(nc) as tc:
        with contextlib.ExitStack() as ctx:
            if repeat > 1:
                ctx.enter_context(tc.For_i(0, repeat, 1))
            const_p = ctx.enter_context(tc.tile_pool(name="const", bufs=1))
            acts_a = ctx.enter_context(tc.tile_pool(name="actsA", bufs=1))
            acts_b = ctx.enter_context(tc.tile_pool(name="actsB", bufs=1))
            xcol_p = ctx.enter_context(tc.tile_pool(name="xcol", bufs=2))
            wconv_p = ctx.enter_context(tc.tile_pool(name="wconv", bufs=2))
            wf5_p = ctx.enter_context(tc.tile_pool(name="wf5s", bufs=24))
            tmp_p = ctx.enter_context(tc.tile_pool(name="tmp", bufs=6))
            fc_p = ctx.enter_context(tc.tile_pool(name="fc", bufs=2))
            ps_conv = ctx.enter_context(
                tc.tile_pool(name="psc", bufs=4, space="PSUM"))
            ps_fc = ctx.enter_context(
                tc.tile_pool(name="psfc", bufs=2, space="PSUM"))
            ps_tp = ctx.enter_context(
                tc.tile_pool(name="pstp", bufs=2, space="PSUM"))

            thr_sb = const_p.tile([128, 30], F32)
            nc.sync.dma_start(thr_sb[:], thr_d[:])
            id_sb = const_p.tile([32, 32], F32)
            nc.sync.dma_start(id_sb[:], id_d[:])
            w1_sb = const_p.tile([27, 128], F32)
            nc.sync.dma_start(w1_sb[:], w1_d[:])
            wf6_sb = const_p.tile([128, 8 * 1024], F8)
            nc.sync.dma_start(wf6_sb[:], wf6_d[:])
            wf7_sb = const_p.tile([128, 80], F8)
            nc.sync.dma_start(wf7_sb[:], wf7_d[:])

            def bias(name, od=0):
                c = tcol[name] + od
                return thr_sb[:, c:c + 1]

            # ---------------- L1: conv1_1, fp32, K=27 im2col ----------------
            s1 = acts_a.tile([128, B * IM1], F8, tag="A")
            s1f0 = s1[:].rearrange("c (b n) -> c b n", b=B)
            nc.gpsimd.memset(s1f0[:, :, 0:34], 0.0)              # row 0
            nc.gpsimd.memset(s1f0[:, :, 33 * 48:33 * 48 + 34], 0.0)  # row 33
            nc.gpsimd.memset(_custom(s1[:, 48:49], [[IM1, B], [48, 32]]), 0.0)
            nc.gpsimd.memset(_custom(s1[:, 48 + 33:48 + 34],
                                     [[IM1, B], [48, 32]]), 0.0)
            s1_v = s1[:].rearrange("c (b h w) -> c b h w", b=B, h=34, w=48)
            GRP = 2
            for g in range(B // GRP):
                xcol = xcol_p.tile([128, GRP * 1156], F32, tag="xcol")
                for t in range(9):
                    dy, dx = t // 3, t % 3
                    off = g * GRP * 1156 + dy * 34 + dx
                    eng = nc.sync if t % 2 == 0 else nc.scalar
                    eng.dma_start(xcol[3 * t:3 * t + 3, :],
                                  xp_d[:, off:off + GRP * 1156])
                xc_v = xcol[:].rearrange("c (b n) -> c b n", b=GRP)
                for li in range(GRP):
                    bi = g * GRP + li
                    for half in range(2):
                        ps = ps_conv.tile([128, 512], F32, tag="ps")
                        rhs = xc_v[0:27, li, half * 544:half * 544 + 544] \
                            .rearrange("c (h w) -> c h w", h=16, w=34)[:, :, 0:32]
                        nc.tensor.matmul(
                            ps[:].rearrange("c (h w) -> c h w", h=16, w=32),
                            w1_sb[:], rhs, start=True, stop=True)
                        dst = s1_v[:, bi, 1 + half * 16:1 + half * 16 + 16, 1:33]
                        nc.scalar.activation(
                            dst, ps[:].rearrange("c (h w) -> c h w", h=16, w=32),
                            mybir.ActivationFunctionType.Sign, bias=bias("1_1"))

            # ---------------- L2: conv1_2 (128->128, 32x32, pool) -----------
            # DR over vertical tap pairs (dy=0,1); dy=2 as normal matmuls.
            w2 = wconv_p.tile([128, WSZ["1_2"]], F8, tag="w")
            nc.sync.dma_start(w2[:], wc_d[:, WOFF["1_2"]:WOFF["1_2"] + WSZ["1_2"]])
            s2 = acts_b.tile([128, B * IM2], F8, tag="B")
            s2f0 = s2[:].rearrange("c (b n) -> c b n", b=B)
            nc.gpsimd.memset(s2f0[:, :, 0:18], 0.0)              # row 0
            nc.gpsimd.memset(s2f0[:, :, 17 * 32:17 * 32 + 18], 0.0)  # row 17
            nc.gpsimd.memset(_custom(s2[:, 32:33], [[IM2, B], [32, 16]]), 0.0)
            nc.gpsimd.memset(_custom(s2[:, 32 + 17:32 + 18],
                                     [[IM2, B], [32, 16]]), 0.0)
            s2_v = s2[:].rearrange("c (b h w) -> c b h w", b=B, h=18, w=32)
            for g in range(B):
                for half in range(2):
                    y0 = half * 16
                    ps = ps_conv.tile([128, 512], F32, tag="ps")
                    psv = ps[:].rearrange("c (h w) -> c h w", h=16, w=32)
                    for dx in range(3):
                        base = s1[:, g * IM1 + y0 * 48 + dx:
                                  g * IM1 + y0 * 48 + dx + 1]
                        rhs = _custom(base, [[48, 2], [48, 16], [1, 32]])
                        nc.tensor.matmul(
                            psv, w2[:, dx * 256:dx * 256 + 256]
                            .rearrange("c (t m) -> c t m", t=2),
                            rhs, start=(dx == 0), stop=False, perf_mode=DR)
                    for dx in range(3):
                        rhs = s1_v[:, g, y0 + 2:y0 + 18, dx:dx + 32]
                        nc.tensor.matmul(
                            psv, w2[:, 768 + dx * 128:768 + dx * 128 + 128],
                            rhs, start=False, stop=(dx == 2))
                    # pool 2x2 on integers, then threshold-sign
                    t1 = tmp_p.tile([128, 256], F32, tag="t1")
                    nc.vector.tensor_reduce(
                        t1[:].rearrange("c (m x) -> c m x", m=16, x=16),
                        ps[:].rearrange("c (m x t) -> c m x t", m=16, x=16, t=2),
                        mybir.AxisListType.X, mybir.AluOpType.max)
                    t2 = tmp_p.tile([128, 128], F32, tag="t2")
                    nc.vector.tensor_reduce(
                        t2[:].rearrange("c (m x) -> c m x", m=8, x=16),
                        t1[:].rearrange("c (m t x) -> c m x t", m=8, t=2, x=16),
                        mybir.AxisListType.X, mybir.AluOpType.max)
                    dst = s2_v[:, g, 1 + half * 8:1 + half * 8 + 8, 1:17]
                    nc.scalar.activation(
                        dst, t2[:].rearrange("c (h w) -> c h w", h=8, w=16),
                        mybir.ActivationFunctionType.Sign, bias=bias("1_2"))

            # ---------------- L3: conv2_1 (128->256, 16x16) -----------------
            w3 = wconv_p.tile([128, WSZ["2_1"]], F8, tag="w")
            nc.sync.dma_start(w3[:], wc_d[:, WOFF["2_1"]:WOFF["2_1"] + WSZ["2_1"]])
            s3 = acts_a.tile([128, 2 * B * IM3], F8, tag="A")
            s3f0 = s3[:].rearrange("c (q n) -> c q n", q=2 * B)
            nc.gpsimd.memset(s3f0[:, :, 0:18], 0.0)
            nc.gpsimd.memset(s3f0[:, :, 17 * 18:18 * 18], 0.0)
            nc.gpsimd.memset(_custom(s3[:, 18:19], [[IM3, 2 * B], [18, 16]]), 0.0)
            nc.gpsimd.memset(_custom(s3[:, 18 + 17:18 + 18],
                                     [[IM3, 2 * B], [18, 16]]), 0.0)
            s3_v = s3[:].rearrange("c (q b h w) -> c q b h w",
                                   q=2, b=B, h=18, w=18)
            s2f = s2[:]
            for g in range(B):
                for od in range(2):
                    wo = od * (3 * 2 * 128 + 3 * 128)
                    ps = ps_conv.tile([128, 256], F32, tag="ps")
                    psv = ps[:].rearrange("c (h w) -> c h w", h=16, w=16)
                    for dx in range(3):
                        base = s2f[:, g * IM2 + dx:g * IM2 + dx + 1]
                        rhs = _custom(base, [[32, 2], [32, 16], [1, 16]])
                        nc.tensor.matmul(
                            psv, w3[:, wo + dx * 256:wo + dx * 256 + 256]
                            .rearrange("c (t m) -> c t m", t=2),
                            rhs, start=(dx == 0), stop=False, perf_mode=DR)
                    for dx in range(3):
                        rhs = s2_v[:, g, 2:18, dx:dx + 16]
                        nc.tensor.matmul(
                            psv, w3[:, wo + 768 + dx * 128:wo + 768 + dx * 128 + 128],
                            rhs, start=False, stop=(dx == 2))
                    dst = s3_v[:, od, g, 1:17, 1:17]
                    nc.scalar.activation(
                        dst, psv, mybir.ActivationFunctionType.Sign,
                        bias=bias("2_1", od))

            # ---------------- L4: conv2_2 (256->256, 16x16, pool) -----------
            # DR over cin pairs; per-image interior chunks N=256.
            w4 = wconv_p.tile([128, WSZ["2_2"]], F8, tag="w")
            nc.sync.dma_start(w4[:], wc_d[:, WOFF["2_2"]:WOFF["2_2"] + WSZ["2_2"]])
            s4 = acts_b.tile([128, 2 * CSLACK + 2 * CV], F8, tag="B")

            def canvas_memset(s, ncd):
                L = 2 * CSLACK + ncd * CV
                nc.gpsimd.memset(s[:, 0:CSLACK], 0.0)
                nc.gpsimd.memset(s[:, L - CSLACK:L], 0.0)
                for cd in range(ncd):
                    b0 = CSLACK + cd * CV
                    # 5 border rows (0,9,18,27,36), 9 border cols, 3 pad elems
                    nc.gpsimd.memset(
                        _custom(s[:, b0:b0 + 1], [[9 * CV_C, 5], [1, CV_C]]), 0.0)
                    nc.gpsimd.memset(
                        _custom(s[:, b0:b0 + 1], [[CV_C, CV_R], [9, 9]]), 0.0)
                    nc.gpsimd.memset(s[:, b0 + 2701:b0 + 2704], 0.0)

            canvas_memset(s4, 2)
            for g in range(B):
                for od in range(2):
                    ps = ps_conv.tile([128, 256], F32, tag="ps")
                    psv = ps[:].rearrange("c (h w) -> c h w", h=16, w=16)
                    for t in range(9):
                        dy, dx = t // 3, t % 3
                        wo = (od * 9 + t) * 256
                        rhs = s3_v[:, :, g, dy:dy + 16, dx:dx + 16]
                        nc.tensor.matmul(
                            psv, w4[:, wo:wo + 256]
                            .rearrange("c (t m) -> c t m", t=2),
                            rhs, start=(t == 0), stop=(t == 8), perf_mode=DR)
                    t1 = tmp_p.tile([128, 128], F32, tag="t1")
                    nc.vector.tensor_reduce(
                        t1[:, 0:128].rearrange("c (m x) -> c m x", m=16, x=8),
                        ps[:].rearrange("c (m x t) -> c m x t", m=16, x=8, t=2),
                        mybir.AxisListType.X, mybir.AluOpType.max)
                    t2 = tmp_p.tile([128, 64], F32, tag="t2")
                    nc.vector.tensor_reduce(
                        t2[:, 0:64].rearrange("c (m x) -> c m x", m=8, x=8),
                        t1[:, 0:128].rearrange("c (m t x) -> c m x t",
                                               m=8, t=2, x=8),
                        mybir.AxisListType.X, mybir.AluOpType.max)
                    ir, ic = g // 8, g % 8
                    b4 = (CSLACK + od * CV + (1 + 9 * ir) * CV_C + 1 + 9 * ic)
                    dst = _custom(s4[:, b4:b4 + 1], [[CV_C, 8], [1, 8]])
                    nc.scalar.activation(
                        dst, t2[:, 0:64].rearrange("c (h w) -> c h w", h=8, w=8),
                        mybir.ActivationFunctionType.Sign, bias=bias("2_2", od))

            # ---------------- L5: conv3_1 (256->512, 8x8) -------------------
            # DR over cin pairs; 6-canvas-row chunks N=438 (borders junk).
            w5 = wconv_p.tile([128, WSZ["3_1"]], F8, tag="w")
            nc.sync.dma_start(w5[:], wc_d[:, WOFF["3_1"]:WOFF["3_1"] + WSZ["3_1"]])
            s5 = acts_a.tile([128, 2 * CSLACK + 4 * CV], F8, tag="A")
            canvas_memset(s5, 4)
            SEGS = {0: [(1, 6)], 1: [(7, 2), (10, 3)], 2: [(13, 5)],
                    3: [(19, 6)], 4: [(25, 2), (28, 3)], 5: [(31, 5)]}
            for c6 in range(6):
                r0 = 1 + 6 * c6
                for od in range(4):
                    ps = ps_conv.tile([128, 438], F32, tag="ps")
                    for t in range(9):
                        dy, dx = t // 3, t % 3
                        wo = (od * 9 + t) * 256
                        boff = (CSLACK + r0 * CV_C + (dy - 1) * CV_C + (dx - 1))
                        rhs = _custom(s4[:, boff:boff + 1], [[CV, 2], [1, 438]])
                        nc.tensor.matmul(
                            ps[:], w5[:, wo:wo + 256]
                            .rearrange("c (t m) -> c t m", t=2),
                            rhs, start=(t == 0), stop=(t == 8), perf_mode=DR)
                    for rs, nr in SEGS[c6]:
                        src_ = _custom(ps[:, (rs - r0) * CV_C + 1:
                                       (rs - r0) * CV_C + 2],
                                       [[CV_C, nr], [9, 8], [1, 8]])
                        dpos = CSLACK + od * CV + rs * CV_C + 1
                        dst = _custom(s5[:, dpos:dpos + 1],
                                      [[CV_C, nr], [9, 8], [1, 8]])
                        nc.scalar.activation(
                            dst, src_, mybir.ActivationFunctionType.Sign,
                            bias=bias("3_1", od))

            # ---------------- L6: conv3_2 (512->512, 8x8, pool) -------------
            # 4-canvas-row chunks aligned to image rows, N=292.
            w6 = wconv_p.tile([128, WSZ["3_2"]], F8, tag="w")
            nc.sync.dma_start(w6[:], wc_d[:, WOFF["3_2"]:WOFF["3_2"] + WSZ["3_2"]])
            s6 = acts_b.tile([128, 4 * B * 16], F8, tag="B")
            for ir in range(4):
                for h in range(2):
                    r0 = 9 * ir + 1 + 4 * h
                    for od in range(4):
                        ps = ps_conv.tile([128, 292], F32, tag="ps")
                        mi = 0
                        for pp in range(2):
                            for t in range(9):
                                dy, dx = t // 3, t % 3
                                wo = ((od * 9 + t) * 2 + pp) * 256
                                boff = (CSLACK + pp * 2 * CV + r0 * CV_C
                                        + (dy - 1) * CV_C + (dx - 1))
                                rhs = _custom(s5[:, boff:boff + 1],
                                              [[CV, 2], [1, 292]])
                                nc.tensor.matmul(
                                    ps[:], w6[:, wo:wo + 256]
                                    .rearrange("c (t m) -> c t m", t=2),
                                    rhs, start=(mi == 0), stop=(mi == 17),
                                    perf_mode=DR)
                                mi += 1
                        # pool 2x2: per-row x-pairs, then y-pairs merged
                        t1 = tmp_p.tile([128, 128], F32, tag="t1")
                        for j in range(4):
                            src1 = _custom(ps[:, j * CV_C + 1:j * CV_C + 2],
                                           [[9, 8], [2, 4], [1, 2]])
                            nc.vector.tensor_reduce(
                                t1[:, j * 32:j * 32 + 32]
                                .rearrange("c (m x) -> c m x", m=8, x=4),
                                src1, mybir.AxisListType.X, mybir.AluOpType.max)
                        t2 = tmp_p.tile([128, 64], F32, tag="t2")
                        nc.vector.tensor_reduce(
                            t2[:, 0:64].rearrange("c (m x) -> c m x", m=2, x=32),
                            t1[:].rearrange("c (m t x) -> c m x t",
                                            m=2, t=2, x=32),
                            mybir.AxisListType.X, mybir.AluOpType.max)
                        base6 = (od * B + ir * 8) * 16 + 8 * h
                        dst = _custom(s6[:, base6:base6 + 1],
                                      [[4, 2], [16, 8], [1, 4]])
                        nc.scalar.activation(
                            dst, t2[:, 0:64].rearrange("c (r i x) -> c r i x",
                                                       r=2, i=8, x=4),
                            mybir.ActivationFunctionType.Sign,
                            bias=bias("3_2", od))

            # ---------------- FC head ----------------
            # fc5 with DR: lhsT = s6 cd-pairs [128,2,32], rhs = streamed W5
            ps5a = ps_fc.tile([32, 512], F32, tag="fc")
            ps5b = ps_fc.tile([32, 512], F32, tag="fc")
            s6_v = s6[:].rearrange("c (p t b n) -> c p t b n", p=2, t=2, b=B)
            for kk in range(32):
                cdp, yx = kk // 16, kk % 16
                w5t = wf5_p.tile([128, 2048], F8, tag="w5")
                nc.sync.dma_start(w5t[:], wf5_d[:, kk * 2048:(kk + 1) * 2048])
                lhsT = s6_v[:, cdp, :, :, yx]        # [128, 2, 32]
                w5v = w5t[:].rearrange("c (t n) -> c t n", t=2)
                nc.tensor.matmul(ps5a[:], lhsT, w5v[:, :, 0:512],
                                 start=(kk == 0), stop=(kk == 31), perf_mode=DR)
                nc.tensor.matmul(ps5b[:], lhsT, w5v[:, :, 512:1024],
                                 start=(kk == 0), stop=(kk == 31), perf_mode=DR)
            h5 = fc_p.tile([32, 1024], F32, tag="h")
            nc.vector.tensor_copy(h5[:, 0:512], ps5a[:])
            nc.vector.tensor_copy(h5[:, 512:1024], ps5b[:])

            s5t = fc_p.tile([128, 8 * 32], F8, tag="st")
            for c in range(8):
                pt = ps_tp.tile([128, 32], F32, tag="tp")
                nc.tensor.transpose(pt[:], h5[:, c * 128:(c + 1) * 128], id_sb[:])
                nc.scalar.activation(
                    s5t[:, c * 32:(c + 1) * 32], pt[:],
                    mybir.ActivationFunctionType.Sign,
                    bias=thr_sb[:, tcol["5"] + c:tcol["5"] + c + 1])

            ps6a = ps_fc.tile([32, 512], F32, tag="fc")
            ps6b = ps_fc.tile([32, 512], F32, tag="fc")
            for c in range(8):
                lhsT = s5t[:, c * 32:(c + 1) * 32]
                nc.tensor.matmul(ps6a[:], lhsT, wf6_sb[:, c * 1024:c * 1024 + 512],
                                 start=(c == 0), stop=(c == 7))
                nc.tensor.matmul(ps6b[:], lhsT,
                                 wf6_sb[:, c * 1024 + 512:c * 1024 + 1024],
                                 start=(c == 0), stop=(c == 7))
            h6 = fc_p.tile([32, 1024], F32, tag="h")
            nc.vector.tensor_copy(h6[:, 0:512], ps6a[:])
            nc.vector.tensor_copy(h6[:, 512:1024], ps6b[:])

            s6t = fc_p.tile([128, 8 * 32], F8, tag="st")
            for c in range(8):
                pt = ps_tp.tile([128, 32], F32, tag="tp")
                nc.tensor.transpose(pt[:], h6[:, c * 128:(c + 1) * 128], id_sb[:])
                nc.scalar.activation(
                    s6t[:, c * 32:(c + 1) * 32], pt[:],
                    mybir.ActivationFunctionType.Sign,
                    bias=thr_sb[:, tcol["6"] + c:tcol["6"] + c + 1])

            ps7 = ps_tp.tile([32, 10], F32, tag="tp")
            for c in range(8):
                nc.tensor.matmul(ps7[:], s6t[:, c * 32:(c + 1) * 32],
                                 wf7_sb[:, c * 10:(c + 1) * 10],
                                 start=(c == 0), stop=(c == 7))
            o_sb = const_p.tile([32, 10], F32)
            nc.vector.tensor_copy(o_sb[:], ps7[:])
            nc.sync.dma_start(out_d[:], o_sb[:])

    nc.compile()
    return nc


def prep_inputs(x, params):
    x = np.asarray(x, np.float32)
    p = {k: np.asarray(v) for k, v in params.items()}

    xs = x.reshape(N_CORES, B, 32, 32, 3)
    xp = np.zeros((N_CORES, 3, B, 34, 34), np.float32)
    xp[:, :, :, 1:33, 1:33] = xs.transpose(0, 4, 1, 2, 3)
    xp_flat = np.zeros((N_CORES, 3, XP_LEN), np.float32)
    xp_flat[:, :, :B * 1156] = xp.reshape(N_CORES, 3, -1)

    w1 = np.ascontiguousarray(
        _sgn(p["w_conv_1_1"]).reshape(27, 128)).astype(np.float32)

    wc = np.zeros((128, WC_TOT), NP8)

    def put(name, arr):
        wc[:, WOFF[name]:WOFF[name] + WSZ[name]] = \
            np.ascontiguousarray(arr.reshape(128, -1)).astype(NP8)

    # conv1_2: [dy,dx,cm,om] -> DR [cm,(dx,2,om)] + N [cm,(dx,om)]
    a = _sgn(p["w_conv_1_2"]).reshape(3, 3, 128, 128)
    put("1_2", np.concatenate(
        [a[0:2].transpose(2, 1, 0, 3).reshape(128, -1),
         a[2].transpose(1, 0, 2).reshape(128, -1)], axis=1))
    # conv2_1: [dy,dx,cm,od,om] -> per od: DR + N
    a = _sgn(p["w_conv_2_1"]).reshape(3, 3, 128, 2, 128)
    blocks = []
    for od in range(2):
        blocks.append(a[0:2, :, :, od].transpose(2, 1, 0, 3).reshape(128, -1))
        blocks.append(a[2, :, :, od].transpose(1, 0, 2).reshape(128, -1))
    put("2_1", np.concatenate(blocks, axis=1))
    # conv2_2: [dy,dx,cd2,cm,od2,om] -> [cm,(od,t,cd,om)]
    a = _sgn(p["w_conv_2_2"]).reshape(3, 3, 2, 128, 2, 128)
    put("2_2", a.transpose(3, 4, 0, 1, 2, 5).reshape(128, -1))
    # conv3_1: [dy,dx,cd2,cm,od4,om] -> [cm,(od,t,cd,om)]
    a = _sgn(p["w_conv_3_1"]).reshape(3, 3, 2, 128, 4, 128)
    put("3_1", a.transpose(3, 4, 0, 1, 2, 5).reshape(128, -1))
    # conv3_2: [dy,dx,pp2,i2,cm,od4,om] -> [cm,(od,t,pp,i,om)]
    a = _sgn(p["w_conv_3_2"]).reshape(3, 3, 2, 2, 128, 4, 128)
    put("3_2", a.transpose(4, 5, 0, 1, 2, 3, 6).reshape(128, -1))

    # fc5: rows f=(y,x,cd,cm); pair (2cdp, 2cdp+1) -> [cm,(cdp,y,x,i,o)]
    w5 = _sgn(p["w_fc_5"]).reshape(4, 4, 2, 2, 128, 1024)  # (y,x,cdp,i,cm,o)
    wf5 = np.ascontiguousarray(
        w5.transpose(4, 2, 0, 1, 3, 5).reshape(128, -1)).astype(NP8)
    w6 = _sgn(p["w_fc_6"]).reshape(8, 128, 1024)
    wf6 = np.ascontiguousarray(w6.transpose(1, 0, 2).reshape(128, -1)).astype(NP8)
    w7 = _sgn(p["w_fc_7"]).reshape(8, 128, 10)
    wf7 = np.ascontiguousarray(w7.transpose(1, 0, 2).reshape(128, -1)).astype(NP8)

    thr = np.zeros((128, 30), np.float32)
    thr[:, 0] = -_thresh(p, "bn_1_1").astype(np.float32)
    thr[:, 1] = -_odd_thresh(p, "bn_1_2")
    thr[:, 2:4] = -_odd_thresh(p, "bn_2_1").reshape(2, 128).T
    thr[:, 4:6] = -_odd_thresh(p, "bn_2_2").reshape(2, 128).T
    thr[:, 6:10] = -_odd_thresh(p, "bn_3_1").reshape(4, 128).T
    thr[:, 10:14] = -_odd_thresh(p, "bn_3_2").reshape(4, 128).T
    thr[:, 14:22] = -_odd_thresh(p, "bn_5").reshape(8, 128).T
    thr[:, 22:30] = -_odd_thresh(p, "bn_6").reshape(8, 128).T

    ident = np.eye(32, dtype=np.float32)

    shared = {"w1": w1, "wc": wc, "wf5": wf5, "wf6": wf6, "wf7": wf7,
              "thr": thr, "ident": ident}
    in_maps = [dict(shared, xp=xp_flat[c]) for c in range(N_CORES)]
    return in_maps


def postprocess(h7_all, params):
    p = params
    m = np.asarray(p["bn_7_mean"], np.float32)
    v = np.asarray(p["bn_7_var"], np.float32)
    b = np.asarray(p["bn_7_beta"], np.float32)
    y = (h7_all - m) * (np.float32(1.0) / np.sqrt(v + np.float32(BN_EPS))) + b
    y = y - y.max(axis=-1, keepdims=True)
    e = np.exp(y)
    return (e / e.sum(axis=-1, keepdims=True)).astype(np.float32)


_NC_CACHE = {}


def get_nc():
    if "nc" not in _NC_CACHE:
        _NC_CACHE["nc"] = build_program()
    return _NC_CACHE["nc"]


def kernel(x, params):
    params = {k: np.asarray(v) for k, v in params.items()}
    nc = get_nc()
    in_maps = prep_inputs(x, params)
    res = run_bass_kernel_spmd(nc, in_maps, list(range(N_CORES)))
    h7 = np.concatenate([res.results[c]["out"] for c in range(N_CORES)], axis=0)
    return postprocess(h7, params)
